# revision 41
# baseline (speedup 1.0000x reference)
"""Distributed Trainium2 Bass kernel for nn_AddModelWithAttentionStacked.

Sharding: mesh B(4) x L(2) over 8 NeuronCores. Core c owns batch b=c//2 and
sequence rows [r0, r0+256) with r0 = (c%2)*256. Activations are kept
feature-major (E on partitions) in SBUF.

Rows are kept in per-core [mine | remote] order (own 256 rows first, then the
other half's 256 rows). Since the two halves are cyclically adjacent both
ways, the roll-by-one windows become static slices (boundary column = remote
row 255 / 0 for every core) -- no shift matmuls needed. All row-order
dependent host data (MSEL) is permuted per core.

Per-layer boundary: cores exchange the UNNORMALIZED residual y = x + a*u
plus the per-row scale s2 (packed into the same pair AllGather payload) so
the whole norm chain and the next layer's q-projection overlap the
collective flight time. Norm stats live in [128,2] partition layout (rows on
partitions) so the serial chain runs at ~128x parallelism.

Head: pair-local vocab split; each core computes logits for its OWN batch
over half the vocab, and outputs partial sum-exp + target-logit dots; the
final log-softmax / loss combine happens host-side. No global collectives:
just 6 pair AllGathers + a pair rendezvous.

Matmul compute in bf16 (fp32 accumulation in PSUM); norms and stats in fp32.
"""

import numpy as np
import ml_dtypes

G, E, K, D, B, L, M, KN = 32000, 256, 8, 6, 4, 512, 64, 4
STEP, EPS = 0.05, 1.0
NCORES = 8
RL = L // 2          # 256 local rows
VS = G // 2          # 16000 vocab per core (pair-local split)
VC = 500             # vocab chunk
NVC = VS // VC       # 32

_D_EFF = D
_DEBUG = False
_TRACE = False
_CACHE = {}

bf16np = ml_dtypes.bfloat16
f8np = ml_dtypes.float8_e4m3


def _f8(x):
    return np.ascontiguousarray(np.asarray(x, np.float32).astype(f8np))

PAIRS = [[0, 1], [2, 3], [4, 5], [6, 7]]


def _bf(x):
    return np.ascontiguousarray(np.asarray(x, np.float32).astype(bf16np))


def _f32(x):
    return np.ascontiguousarray(np.asarray(x, np.float32))


def _norm_np(x):
    return x / (EPS + np.std(x, axis=-1, ddof=1, keepdims=True))


def _fm(x):
    """feature-major: (rows, 256) -> [p, ec*rows + j] = x[j, ec*128+p]"""
    r = x.shape[0]
    return x.reshape(r, 2, 128).transpose(2, 1, 0).reshape(128, 2 * r)


def _prep(inputs):
    masked = np.asarray(inputs['masked'])
    unmasked = np.asarray(inputs['unmasked'])
    mask = np.asarray(inputs['mask'])
    summer = np.asarray(inputs['summer'], np.float32)
    embed = np.asarray(inputs['embed'], np.float32)
    pos = np.asarray(inputs['pos'], np.float32)
    Wt = np.asarray(inputs['Wt'], np.float32)
    Wc = np.asarray(inputs['Wc'], np.float32)
    Wq = np.asarray(inputs['Wq'], np.float32)
    Wd = np.asarray(inputs['Wd'], np.float32)
    Wo = np.asarray(inputs['Wo'], np.float32)
    Wkc = np.asarray(inputs['Wkc'], np.float32)
    bkc = np.asarray(inputs['bkc'], np.float32)
    Wem = np.asarray(inputs['Wem'], np.float32)

    # ---- shared (identical on all cores) ----
    def blk_nat(w):  # w (D, 256, 256): [d, p, kc, mc, c] = w[d, kc*128+p, mc*128+c]
        return w.reshape(D, 2, 128, 2, 128).transpose(0, 2, 1, 3, 4)

    def blk_tr(w):   # [d, p, kc, mc, c] = w[d, mc*128+c, kc*128+p]
        return w.reshape(D, 2, 128, 2, 128).transpose(0, 4, 3, 1, 2)

    wtc = np.stack([blk_nat(Wt), blk_nat(Wc), blk_tr(Wc), blk_tr(Wt)], axis=2)
    WTC = _bf(wtc.reshape(D, 128, 4 * 2 * 2 * 128))

    # WQT: [d, p, kc(2), mc(16), c] = Wq[d, mc*128+c, kc*128+p]
    wq = Wq.reshape(D, 16, 128, 2, 128).transpose(0, 4, 3, 1, 2)
    WQT = _bf(wq.reshape(D, 128, 2 * 16 * 128))

    # WDT: [d, kc(16), p, mc(16), c] = Wd[d, mc*128+c, kc*128+p]
    wd = Wd.reshape(D, 16, 128, 16, 128).transpose(0, 4, 3, 1, 2)
    wd = wd.transpose(0, 2, 1, 3, 4)
    WDT = _f8(wd.reshape(D, 16, 128, 16 * 128))

    # WO: [d, p, kc(16), mc(2), c] = Wo[d, kc*128+p, mc*128+c]
    wo = Wo.reshape(D, 16, 128, 2, 128).transpose(0, 2, 1, 3, 4)
    WO = _bf(wo.reshape(D, 128, 16 * 2 * 128))

    # WKCT: [p, fc(2), knec(8), c] = Wkc[knec*128+c, fc*128+p]
    wk = Wkc.reshape(8, 128, 2, 128).transpose(3, 2, 0, 1)
    WKCT = _bf(wk.reshape(128, 2 * 8 * 128))

    # WEM: [p, kc(2), ec(2), c] = Wem[kc*128+p, ec*128+c]
    we = Wem.reshape(2, 128, 2, 128).transpose(1, 0, 2, 3)
    WEM = _bf(we.reshape(128, 2 * 2 * 128))

    BKC = _f32(bkc.reshape(8, 128).T)  # (128, 8) [p, knec]

    # ---- derived host math ----
    xsa0 = _norm_np(embed[masked] + pos[None])  # (B, L, E) f32
    tgt = np.take_along_axis(unmasked, mask, axis=1)  # (B, M)

    # SEL2: [2,256] row-selector for K=2 broadcast matmuls
    sel2 = np.zeros((2, 256), np.float32)
    sel2[0, 0:128] = 1.0
    sel2[1, 128:256] = 1.0

    shared = dict(WTC=WTC, WQT=WQT, WDT=WDT, WO=WO, WKCT=WKCT, WEM=WEM,
                  BKC=BKC, SEL2=_bf(sel2))

    # ---- per-core ----
    in_maps = []
    for c in range(NCORES):
        b, h = c // 2, c % 2
        r0, o0 = h * RL, (1 - h) * RL
        m = dict(shared)
        xb = xsa0[b]  # (512, 256)
        x0 = xb[r0:r0 + RL]
        xr = xb[o0:o0 + RL]
        # XSA0 (master, own rows, f32, feature-major)
        m['XSA0'] = _f32(_fm(x0))
        # XR0 (remote rows, bf16, feature-major)
        m['XR0'] = _bf(_fm(xr))
        # STC0: [p, t] = sum(x0[t*128+p]); [p, 2+t] = sumsq
        s = x0.sum(-1).reshape(2, 128).T
        q = (x0 * x0).sum(-1).reshape(2, 128).T
        m['STC0'] = _f32(np.concatenate([s, q], 1))
        # NAT0 (core-order rows [mine|remote], natural layout)
        xcore = np.concatenate([x0, xr])  # (512, 256)
        m['NAT0'] = _bf(xcore.reshape(4, 128, 2, 128).transpose(1, 0, 2, 3)
                        .reshape(128, 1024))
        # RSEL: remote gather slot selector (slot 1-h is the remote core)
        rs = np.zeros((128, 2), np.float32)
        rs[:, 1 - h] = 1.0
        m['RSEL'] = _f32(rs)
        # MSEL in per-core row order: core-row of global l
        ms = np.zeros((L, M), np.float32)
        gl = mask[b]  # (M,) global rows
        crow = np.where(gl // RL == h, gl - r0, RL + gl - o0)
        ms[crow, np.arange(M)] = 1.0
        m['MSEL'] = _bf(ms.reshape(4, 128, M).transpose(1, 0, 2).reshape(128, 4 * M))
        # ETT (own batch): rows n = m*KN+kn -> embed[tgt[b, m]]
        ett = embed[np.repeat(tgt[b], KN)]  # (256, 256)
        m['ETT'] = _bf(_fm(ett))
        # EMBT (own half-vocab): [vc, p, ec*500+n] = embed[h*VS+vc*500+n, ec*128+p]
        shard = embed[h * VS:(h + 1) * VS]  # (16000, 256)
        et = shard.reshape(NVC, VC, 2, 128).transpose(0, 3, 2, 1)
        m['EMBT'] = _f8(et.reshape(NVC, 128, 2 * VC))
        in_maps.append(m)

    aux = dict(summer=summer)
    return in_maps, aux


def _build(d_eff, debug):
    import concourse.bass as bass
    import concourse.tile as tile
    from concourse import mybir, bacc
    from concourse.masks import make_identity
    from contextlib import ExitStack

    dt = mybir.dt
    AF = mybir.ActivationFunctionType

    nc = bacc.Bacc("TRN2", num_devices=NCORES)

    def par(name, shape, dtype=dt.bfloat16):
        return nc.dram_tensor(name, shape, dtype, kind="ExternalInput")

    P = {}
    P['WTC'] = par('WTC', [D, 128, 2048])
    P['WQT'] = par('WQT', [D, 128, 4096])
    P['WDT'] = par('WDT', [D, 16, 128, 2048], dt.float8e4)
    P['WO'] = par('WO', [D, 128, 4096])
    P['WKCT'] = par('WKCT', [128, 2048])
    P['WEM'] = par('WEM', [128, 512])
    P['BKC'] = par('BKC', [128, 8], dt.float32)
    P['ETT'] = par('ETT', [128, 512])
    P['XSA0'] = par('XSA0', [128, 512], dt.float32)
    P['XR0'] = par('XR0', [128, 512])
    P['STC0'] = par('STC0', [128, 4], dt.float32)
    P['NAT0'] = par('NAT0', [128, 1024])
    P['RSEL'] = par('RSEL', [128, 2], dt.float32)
    P['SEL2'] = par('SEL2', [2, 256])
    P['MSEL'] = par('MSEL', [128, 256])
    P['EMBT'] = par('EMBT', [NVC, 128, 1000], dt.float8e4)

    osum_t = nc.dram_tensor("osum", [128, 2], dt.float32, kind="ExternalOutput")
    oclog_t = nc.dram_tensor("oclog", [1, 256], dt.float32, kind="ExternalOutput")
    dbg = {}

    def dbg_out(name, shape, dtype):
        if debug and name not in dbg:
            dbg[name] = nc.dram_tensor(name, shape, dtype, kind="ExternalOutput")
        return dbg.get(name)

    with tile.TileContext(nc) as tc, ExitStack() as ctx:
        con = ctx.enter_context(tc.tile_pool(name="con", bufs=1))
        pers = ctx.enter_context(tc.tile_pool(name="pers", bufs=1))
        sb = ctx.enter_context(tc.tile_pool(name="sb", bufs=2))
        mpool = ctx.enter_context(tc.tile_pool(name="mpool", bufs=2))
        wdp = ctx.enter_context(tc.tile_pool(name="wdp", bufs=32))
        rows = ctx.enter_context(tc.tile_pool(name="rows", bufs=1))
        hp = ctx.enter_context(tc.tile_pool(name="hp", bufs=1))
        pp = ctx.enter_context(tc.tile_pool(name="pp", bufs=5, space="PSUM"))
        ppx = ctx.enter_context(tc.tile_pool(name="ppx", bufs=1, space="PSUM"))
        pps = ctx.enter_context(tc.tile_pool(name="pps", bufs=2, space="PSUM"))
        dram = ctx.enter_context(tc.tile_pool(name="dram", bufs=2, space="DRAM"))

        mm = nc.tensor.matmul
        act = nc.scalar.activation
        V = nc.vector
        STT = mybir.AluOpType

        # rendezvous: tiny pair all-reduce to absorb core-start skew
        rdv_in = dram.tile([128], dt.float32, tag='rdv_in')
        rdv_out = dram.tile([128], dt.float32, tag='rdv_out')
        rdv_sb = con.tile([1, 128], dt.float32)
        V.memset(rdv_sb, 0.0)
        nc.gpsimd.dma_start(out=rdv_in[:], in_=rdv_sb[:])
        nc.gpsimd.collective_compute(
            "AllReduce", mybir.AluOpType.add,
            replica_groups=PAIRS,
            ins=[rdv_in.opt()], outs=[rdv_out.opt()],
        )

        # initial state -- XSA0 + Wq(0) first: they gate the first matmuls
        master = mpool.tile([128, 512], dt.float32, tag='master')
        nc.scalar.dma_start(out=master[:], in_=P['XSA0'][:])
        wq0 = sb.tile([128, 4096], dt.bfloat16, tag='wq', bufs=2, name='wq')
        nc.scalar.dma_start(out=wq0[:], in_=P['WQT'][0])
        loc = mpool.tile([128, 512], dt.bfloat16, tag='loc')
        V.tensor_copy(out=loc[:], in_=master[:])
        stc = mpool.tile([128, 4], dt.float32, tag='stc')
        nc.sync.dma_start(out=stc[:], in_=P['STC0'][:])
        rem = sb.tile([128, 514], dt.bfloat16, tag='rem', name='rem')
        nc.sync.dma_start(out=rem[:, 0:512], in_=P['XR0'][:])
        nat = sb.tile([128, 1024], dt.bfloat16, tag='nat', name='nat')
        nc.sync.dma_start(out=nat[:], in_=P['NAT0'][:])

        # constants
        ident = con.tile([128, 128], dt.bfloat16)
        make_identity(nc, ident)
        ones_cb = con.tile([128, 1], dt.bfloat16)
        V.memset(ones_cb, 1.0)
        ones_rb = con.tile([1, 128], dt.bfloat16)
        V.memset(ones_rb, 1.0)
        ones_cf = con.tile([128, 1], dt.float32)
        V.memset(ones_cf, 1.0)
        # row-selector for K=2 broadcast matmuls: sel[:, t*128:+128] picks row t
        sel2 = con.tile([2, 256], dt.bfloat16)
        nc.sync.dma_start(out=sel2[:], in_=P['SEL2'][:])

        # persistent inputs for the layer loop
        rsel = pers.tile([128, 2], dt.float32)
        nc.sync.dma_start(out=rsel[:], in_=P['RSEL'][:])

        def load_wq(d):
            wq = sb.tile([128, 4096], dt.bfloat16, tag='wq', bufs=2, name='wq')
            nc.sync.dma_start(out=wq[:], in_=P['WQT'][d])
            return wq

        def qt_proj(wq, loc_t):
            qT = sb.tile([128, 4096], dt.bfloat16, tag='qT', bufs=1, name='qT')
            for m2 in range(8):
                q_ps = pp.tile([128, 512], dt.float32, tag='ps', name='q_ps')
                for i in range(2):
                    mc = m2 * 2 + i
                    for kc in range(2):
                        mm(q_ps[:, i * 256:(i + 1) * 256],
                           wq[:, (kc * 16 + mc) * 128:(kc * 16 + mc + 1) * 128],
                           loc_t[:, kc * 256:(kc + 1) * 256],
                           start=(kc == 0), stop=(kc == 1))
                V.tensor_copy(out=qT[:, m2 * 512:(m2 + 1) * 512], in_=q_ps[:])
            return qT

        def nat_mine(loc_t):
            """new nat tile with own-row blocks (kb 0,1) transposed in"""
            natt = sb.tile([128, 1024], dt.bfloat16, tag='nat', name='nat')
            for t in range(2):
                for ec in range(2):
                    tp = pp.tile([128, 128], dt.bfloat16, tag='ps', name='tp')
                    nc.tensor.transpose(
                        tp[:], loc_t[:, ec * 256 + t * 128: ec * 256 + t * 128 + 128],
                        ident[:])
                    V.tensor_copy(
                        out=natt[:, t * 256 + ec * 128: t * 256 + ec * 128 + 128],
                        in_=tp[:])
            return natt

        qT = qt_proj(wq0, loc)

        def r2(nm):
            return rows.tile([128, 2], dt.float32, tag='r2', bufs=16, name=nm)

        def boundary(bnum, xsad_ps, master_t, loc_t, stc_t, wq_next):
            """gradnorm + residual + layernorm, fused with the pair exchange.

            Sends y = x + a*u (unnormalized) + s2 in one AllGather; returns
            (new master, loc, stc, collective out dram, new nat tile, qT)."""
            xsad_sb = sb.tile([128, 512], dt.float32, tag='xsad_sb', bufs=1,
                              name='xsad_sb')
            act(out=xsad_sb[:], in_=xsad_ps[:], func=AF.Copy)
            sq = sb.tile([128, 512], dt.float32, tag='sq', bufs=1, name='sq')
            act(out=sq[:], in_=xsad_ps[:], func=AF.Square)
            xu = sb.tile([128, 512], dt.float32, tag='xu', bufs=1, name='xu')
            nc.gpsimd.tensor_mul(xu[:], xsad_sb[:], master_t[:])
            # stats in [128,2] rows-on-partitions layout: su, qu, c
            stq = pps.tile([128, 6], dt.float32, tag='pss', name='stq')
            for src, j in ((xsad_sb, 0), (sq, 2), (xu, 4)):
                for t in range(2):
                    for ec in range(2):
                        mm(stq[:, j + t:j + t + 1],
                           src[:, ec * 256 + t * 128: ec * 256 + t * 128 + 128],
                           ones_cf[:], start=(ec == 0), stop=(ec == 1))
            st6 = rows.tile([128, 6], dt.float32, tag='st6', bufs=2, name='st6')
            V.tensor_copy(out=st6[:], in_=stq[:])
            su, qu, cc = st6[:, 0:2], st6[:, 2:4], st6[:, 4:6]
            # alpha = STEP / (1 + std(u))
            t3, t5 = r2('t3'), r2('t5')
            V.scalar_tensor_tensor(out=t3[:], in0=su, scalar=-1.0 / E, in1=su,
                                   op0=STT.mult, op1=STT.mult)
            V.tensor_add(t5[:], t3[:], qu)
            stdu = r2('stdu')
            act(out=stdu[:], in_=t5[:], func=AF.Sqrt, scale=1.0 / (E - 1))
            s1p, s1, alpha = r2('s1p'), r2('s1'), r2('alpha')
            V.tensor_scalar_add(out=s1p[:], in0=stdu[:], scalar1=1.0)
            V.reciprocal(s1[:], s1p[:])
            V.tensor_scalar_mul(out=alpha[:], in0=s1[:], scalar1=STEP)
            # broadcast alpha over features: transpose to a row, outer-product
            alpha_bf = rows.tile([128, 2], dt.bfloat16, tag='r2b', bufs=4,
                                 name='alpha_bf')
            V.tensor_copy(out=alpha_bf[:], in_=alpha[:])
            ta = pps.tile([2, 128], dt.bfloat16, tag='pss', name='ta')
            nc.tensor.transpose(ta[:], alpha_bf[:], ident[:])
            ra = rows.tile([2, 128], dt.bfloat16, tag='ra', bufs=4, name='ra')
            V.tensor_copy(out=ra[:], in_=ta[:])
            bcA_ps = pp.tile([128, 256], dt.float32, tag='ps', name='bcA_ps')
            for t in range(2):
                mm(bcA_ps[:, t * 128:(t + 1) * 128],
                   sel2[:, t * 128:(t + 1) * 128], ra[:],
                   start=True, stop=True)
            bcA = sb.tile([128, 256], dt.float32, tag='bcA', bufs=1, name='bcA')
            act(out=bcA[:], in_=bcA_ps[:], func=AF.Copy)
            # y = x + a*u (f32), cast to bf16 payload
            y = sb.tile([128, 512], dt.float32, tag='y', bufs=1, name='y')
            ybuf = sb.tile([128, 514], dt.bfloat16, tag='ybuf', bufs=1, name='ybuf')
            for ec in range(2):
                eng = V if ec == 0 else nc.gpsimd
                ty = sb.tile([128, 256], dt.float32, tag='tmp', bufs=2, name='ty')
                eng.tensor_mul(ty[:], bcA[:], xsad_sb[:, ec * 256:(ec + 1) * 256])
                eng.tensor_add(y[:, ec * 256:(ec + 1) * 256],
                               master_t[:, ec * 256:(ec + 1) * 256], ty[:])
                eng.tensor_copy(out=ybuf[:, ec * 256:(ec + 1) * 256],
                                in_=y[:, ec * 256:(ec + 1) * 256])
            # s2 = 1 / (1 + std(y))  via carried stats
            asu, sy = r2('asu'), r2('sy')
            V.tensor_mul(asu[:], alpha[:], su)
            V.tensor_add(sy[:], asu[:], stc_t[:, 0:2])
            ac2, aa, aqu, qy0, qy = r2('ac2'), r2('aa'), r2('aqu'), r2('qy0'), r2('qy')
            V.scalar_tensor_tensor(out=ac2[:], in0=alpha[:], scalar=2.0, in1=cc,
                                   op0=STT.mult, op1=STT.mult)
            V.tensor_mul(aa[:], alpha[:], alpha[:])
            V.tensor_mul(aqu[:], aa[:], qu)
            V.tensor_add(qy0[:], stc_t[:, 2:4], ac2[:])
            V.tensor_add(qy[:], qy0[:], aqu[:])
            t4, t5b = r2('t4'), r2('t5b')
            V.scalar_tensor_tensor(out=t4[:], in0=sy[:], scalar=-1.0 / E, in1=sy[:],
                                   op0=STT.mult, op1=STT.mult)
            V.tensor_add(t5b[:], t4[:], qy[:])
            stdy = r2('stdy')
            act(out=stdy[:], in_=t5b[:], func=AF.Sqrt, scale=1.0 / (E - 1))
            s2p, s2 = r2('s2p'), r2('s2')
            V.tensor_scalar_add(out=s2p[:], in0=stdy[:], scalar1=1.0)
            V.reciprocal(s2[:], s2p[:])
            V.tensor_copy(out=ybuf[:, 512:514], in_=s2[:])
            # launch the exchange as soon as the payload is complete
            ag_in = dram.tile([128, 514], dt.bfloat16, tag='ag_in')
            ag_out = dram.tile([2, 128, 514], dt.bfloat16, tag='ag_out')
            nc.gpsimd.dma_start(out=ag_in[:], in_=ybuf[:])
            nc.gpsimd.collective_compute(
                "AllGather", mybir.AluOpType.bypass,
                replica_groups=PAIRS,
                ins=[ag_in.opt()], outs=[ag_out.opt()],
            )
            # carried stats for next layer
            stc_n = mpool.tile([128, 4], dt.float32, tag='stc', name='stc')
            s2q = r2('s2q')
            V.tensor_mul(stc_n[:, 0:2], sy[:], s2[:])
            V.tensor_mul(s2q[:], s2[:], s2[:])
            V.tensor_mul(stc_n[:, 2:4], qy[:], s2q[:])
            # broadcast s2 and produce the normalized local tile
            ts = pps.tile([2, 128], dt.bfloat16, tag='pss', name='ts')
            nc.tensor.transpose(ts[:], ybuf[:, 512:514], ident[:])
            rs_ = rows.tile([2, 128], dt.bfloat16, tag='ra', bufs=4, name='rs')
            V.tensor_copy(out=rs_[:], in_=ts[:])
            bcS_ps = pp.tile([128, 256], dt.float32, tag='ps', name='bcS_ps')
            for t in range(2):
                mm(bcS_ps[:, t * 128:(t + 1) * 128],
                   sel2[:, t * 128:(t + 1) * 128], rs_[:],
                   start=True, stop=True)
            bcS = sb.tile([128, 256], dt.float32, tag='bcS', bufs=1, name='bcS')
            act(out=bcS[:], in_=bcS_ps[:], func=AF.Copy)
            master_n = mpool.tile([128, 512], dt.float32, tag='master', name='master')
            loc_n = mpool.tile([128, 512], dt.bfloat16, tag='loc', name='loc')
            for ec in range(2):
                eng = V if ec == 0 else nc.gpsimd
                eng.tensor_mul(master_n[:, ec * 256:(ec + 1) * 256],
                               y[:, ec * 256:(ec + 1) * 256], bcS[:])
                eng.tensor_copy(out=loc_n[:, ec * 256:(ec + 1) * 256],
                                in_=master_n[:, ec * 256:(ec + 1) * 256])
            # overlap the collective: next-layer q-projection + nat own blocks
            qT_n = qt_proj(wq_next, loc_n) if wq_next is not None else None
            nat_n = nat_mine(loc_n)
            if debug:
                t = dbg_out(f'dbg_xsa{bnum - 1}', [128, 512], dt.float32)
                nc.sync.dma_start(out=t[:], in_=master_n[:])
            return master_n, loc_n, stc_n, ag_out, nat_n, qT_n

        def finish_gather_min(ag_out):
            """masked-read the remote slot; just enough for the score matmuls"""
            g0 = sb.tile([128, 514], dt.bfloat16, tag='g0', name='g0')
            g1 = sb.tile([128, 514], dt.bfloat16, tag='g1', name='g1')
            nc.gpsimd.dma_start(out=g0[:], in_=ag_out[0])
            nc.gpsimd.dma_start(out=g1[:], in_=ag_out[1])
            g = sb.tile([128, 514], dt.bfloat16, tag='rem', name='rem')
            t0 = sb.tile([128, 514], dt.bfloat16, tag='gt', bufs=1, name='gt')
            V.tensor_scalar_mul(out=t0[:], in0=g0[:], scalar1=rsel[:, 0:1])
            V.scalar_tensor_tensor(out=g[:], in0=g1[:], scalar=rsel[:, 1:2],
                                   in1=t0[:], op0=STT.mult, op1=STT.add)
            s2r = rows.tile([128, 2], dt.float32, tag='s2r', bufs=2, name='s2r')
            V.tensor_copy(out=s2r[:], in_=g[:, 512:514])
            s2r16 = rows.tile([128, 2], dt.float32, tag='s2r16', bufs=2,
                              name='s2r16')
            V.tensor_scalar_mul(out=s2r16[:], in0=s2r[:], scalar1=1.0 / 16.0)
            return g, s2r, s2r16

        def finish_gather_rest(g, s2r, nat_t):
            """scaled remote tile (for roll + head) + nat remote blocks"""
            tr = pps.tile([2, 128], dt.bfloat16, tag='pss', name='trr')
            nc.tensor.transpose(tr[:], g[:, 512:514], ident[:])
            rr = rows.tile([2, 128], dt.bfloat16, tag='ra', bufs=4, name='rr')
            V.tensor_copy(out=rr[:], in_=tr[:])
            bcR_ps = pp.tile([128, 256], dt.float32, tag='ps', name='bcR_ps')
            for t in range(2):
                mm(bcR_ps[:, t * 128:(t + 1) * 128],
                   sel2[:, t * 128:(t + 1) * 128], rr[:],
                   start=True, stop=True)
            bcR = sb.tile([128, 256], dt.float32, tag='bcR', bufs=1, name='bcR')
            act(out=bcR[:], in_=bcR_ps[:], func=AF.Copy)
            rem_t = sb.tile([128, 512], dt.bfloat16, tag='rems', bufs=2, name='rems')
            for ec in range(2):
                eng = V if ec == 0 else nc.gpsimd
                eng.tensor_mul(rem_t[:, ec * 256:(ec + 1) * 256],
                               g[:, ec * 256:(ec + 1) * 256], bcR[:])
            # nat remote blocks: transpose unscaled, scale per-partition on copy
            for t in range(2):
                for ec in range(2):
                    tp = pp.tile([128, 128], dt.bfloat16, tag='ps', name='tpr')
                    nc.tensor.transpose(
                        tp[:], g[:, ec * 256 + t * 128: ec * 256 + t * 128 + 128],
                        ident[:])
                    act(out=nat_t[:, (2 + t) * 256 + ec * 128:
                                  (2 + t) * 256 + ec * 128 + 128],
                        in_=tp[:], func=AF.Copy, scale=s2r[:, t:t + 1])
            return rem_t

        def head_score_half(h, est, half, keys, qT_t):
            s_ps = pp.tile([128, 512], dt.float32, tag='ps', name='s_ps')
            for i in range(2):
                for kc in range(2):
                    mm(s_ps[:, i * 256:(i + 1) * 256],
                       keys[:, kc * 256 + i * 128: kc * 256 + i * 128 + 128],
                       qT_t[:, (h * 2 + kc) * 256:(h * 2 + kc + 1) * 256],
                       start=(kc == 0), stop=(kc == 1))
            act(out=est[:, half * 512:(half + 1) * 512], in_=s_ps[:],
                func=AF.Exp, scale=1.0 / 16.0)

        for d in range(d_eff):
            # pre-gather: local (own-rows) score halves for all heads keep the
            # PE busy during the collective flight
            ests = []
            for h in range(8):
                est = sb.tile([128, 1024], dt.bfloat16, tag='est', bufs=8,
                              name='est')
                head_score_half(h, est, 0, loc, qT)
                ests.append(est)
            if d > 0:
                g, s2r, s2r16 = finish_gather_min(ag_out)

            # --- weight loads (overlap downstream compute) ---
            wtc = sb.tile([128, 2048], dt.bfloat16, tag='wtc', bufs=1, name='wtc')
            nc.sync.dma_start(out=wtc[:], in_=P['WTC'][d])
            wdt = []
            for kc in range(16):
                w = wdp.tile([128, 2048], dt.float8e4, tag='wd', name=f'wd{kc}')
                if d == 0:
                    # tiny artificial dep: delay layer-0 Wd streams until the
                    # startup-critical q-projection inputs have landed
                    V.tensor_copy(out=w[0:1, 0:2], in_=qT[0:1, 0:2])
                nc.sync.dma_start(out=w[:], in_=P['WDT'][d, kc])
                wdt.append(w)
            wo = sb.tile([128, 4096], dt.bfloat16, tag='wo', bufs=1, name='wo')
            nc.sync.dma_start(out=wo[:], in_=P['WO'][d])
            wq_next = load_wq(d + 1) if d + 1 < d_eff else None

            # --- attention heads (software-pipelined); the remote score half
            # of head h reads the UNNORMALIZED remote y, folding s2/16 into
            # the per-partition Exp scale, so the PE restarts as soon as the
            # masked-read is done ---
            xid = sb.tile([128, 4096], dt.bfloat16, tag='xid', bufs=1, name='xid')

            def head_front(h):
                est = ests[h]
                if d == 0:
                    head_score_half(h, est, 1, rem, qT)
                else:
                    s_ps = pp.tile([128, 512], dt.float32, tag='ps', name='s_ps')
                    for i in range(2):
                        for kc in range(2):
                            mm(s_ps[:, i * 256:(i + 1) * 256],
                               g[:, kc * 256 + i * 128: kc * 256 + i * 128 + 128],
                               qT[:, (h * 2 + kc) * 256:(h * 2 + kc + 1) * 256],
                               start=(kc == 0), stop=(kc == 1))
                    for i in range(2):
                        act(out=est[:, (2 + i) * 256:(3 + i) * 256],
                            in_=s_ps[:, i * 256:(i + 1) * 256],
                            func=AF.Exp, scale=s2r16[:, i:i + 1])
                sum_ps = pps.tile([1, 256], dt.float32, tag='pss', name='sum_ps')
                for kb in range(4):
                    mm(sum_ps[:], ones_cb[:], est[:, kb * 256:(kb + 1) * 256],
                       start=(kb == 0), stop=(kb == 3))
                rec = rows.tile([1, 256], dt.float32, tag='rec', bufs=2, name='rec')
                V.reciprocal(rec[:], sum_ps[:])
                rec2 = rows.tile([1, 512], dt.bfloat16, tag='rec2', bufs=2,
                                 name='rec2')
                V.tensor_copy(out=rec2[:, 0:256], in_=rec[:])
                V.tensor_copy(out=rec2[:, 256:512], in_=rec[:])
                return est, rec2

            prev = head_front(0)
            if d > 0:
                rem = finish_gather_rest(g, s2r, nat)

            # --- rolled windows: static slices + one remote boundary column ---
            rolled = {}
            for nm in ('p1', 'm1'):
                rt = sb.tile([128, 512], dt.bfloat16, tag=f'r{nm}', bufs=1, name=f'r{nm}')
                for ec in range(2):
                    o = ec * 256
                    if nm == 'p1':
                        V.tensor_copy(out=rt[:, o:o + 1], in_=rem[:, o + 255:o + 256])
                        V.tensor_copy(out=rt[:, o + 1:o + 256], in_=loc[:, o:o + 255])
                    else:
                        V.tensor_copy(out=rt[:, o + 255:o + 256], in_=rem[:, o:o + 1])
                        V.tensor_copy(out=rt[:, o:o + 255], in_=loc[:, o + 1:o + 256])
                rolled[nm] = rt

            # --- local transition terms, accumulated into xsad psum ---
            xsad_ps = ppx.tile([128, 512], dt.float32, tag='xsad', name='xsad_ps')

            def wtc_blk(mat, kc, mc):
                off = ((mat * 2 + kc) * 2 + mc) * 128
                return wtc[:, off:off + 128]

            a1 = sb.tile([128, 512], dt.bfloat16, tag='a1', bufs=1, name='a1')
            a_ps = pp.tile([128, 512], dt.float32, tag='ps', name='a_ps')
            for mc in range(2):
                for kc in range(2):
                    mm(a_ps[:, mc * 256:(mc + 1) * 256], wtc_blk(0, kc, mc),
                       rolled['p1'][:, kc * 256:(kc + 1) * 256],
                       start=(kc == 0), stop=(kc == 1))
            act(out=a1[:], in_=a_ps[:], func=AF.Relu)
            for mc in range(2):
                for kc in range(2):
                    mm(xsad_ps[:, mc * 256:(mc + 1) * 256], wtc_blk(1, kc, mc),
                       a1[:, kc * 256:(kc + 1) * 256],
                       start=(mc == 0 and kc == 0), stop=False)
            a2 = sb.tile([128, 512], dt.bfloat16, tag='a2', bufs=1, name='a2')
            a_ps = pp.tile([128, 512], dt.float32, tag='ps', name='a_ps2')
            for mc in range(2):
                for kc in range(2):
                    mm(a_ps[:, mc * 256:(mc + 1) * 256], wtc_blk(2, kc, mc),
                       rolled['m1'][:, kc * 256:(kc + 1) * 256],
                       start=(kc == 0), stop=(kc == 1))
            act(out=a2[:], in_=a_ps[:], func=AF.Relu)
            for mc in range(2):
                for kc in range(2):
                    mm(xsad_ps[:, mc * 256:(mc + 1) * 256], wtc_blk(3, kc, mc),
                       a2[:, kc * 256:(kc + 1) * 256],
                       start=False, stop=False)

            def head_back(h, est, rec2):
                bc_ps = pp.tile([128, 512], dt.float32, tag='ps', name='bc_ps')
                mm(bc_ps[:], ones_rb[:], rec2[:])
                bc_sb = sb.tile([128, 512], dt.bfloat16, tag='bc_sb', name='bc_sb')
                act(out=bc_sb[:], in_=bc_ps[:], func=AF.Copy)
                y_ps = pp.tile([128, 512], dt.float32, tag='ps', name='y_ps')
                for ec in range(2):
                    for kb in range(4):
                        mm(y_ps[:, ec * 256:(ec + 1) * 256],
                           nat[:, kb * 256 + ec * 128: kb * 256 + ec * 128 + 128],
                           est[:, kb * 256:(kb + 1) * 256],
                           start=(kb == 0), stop=(kb == 3))
                V.tensor_mul(xid[:, h * 512:(h + 1) * 512], y_ps[:], bc_sb[:])

            for h in range(1, 8):
                cur = head_front(h)
                head_back(h - 1, *prev)
                prev = cur
            head_back(7, *prev)

            # --- dense relu (Wd) ---
            actb = sb.tile([128, 4096], dt.bfloat16, tag='actb', bufs=1, name='actb')
            for m2 in range(8):
                act_ps = pp.tile([128, 512], dt.float32, tag='ps', name='act_ps')
                for i in range(2):
                    mc = m2 * 2 + i
                    for kc in range(16):
                        mm(act_ps[:, i * 256:(i + 1) * 256],
                           wdt[kc][:, mc * 128:(mc + 1) * 128],
                           xid[:, kc * 256:(kc + 1) * 256],
                           start=(kc == 0), stop=(kc == 15))
                act(out=actb[:, m2 * 512:(m2 + 1) * 512], in_=act_ps[:],
                    func=AF.Relu)

            # --- Wo accumulate into xsad ---
            for mc in range(2):
                for kc in range(16):
                    mm(xsad_ps[:, mc * 256:(mc + 1) * 256],
                       wo[:, (kc * 2 + mc) * 128:(kc * 2 + mc + 1) * 128],
                       actb[:, kc * 256:(kc + 1) * 256],
                       start=False, stop=(mc == 1 and kc == 15))

            # --- boundary: norm + exchange + next-layer prologue ---
            master, loc, stc, ag_out, nat, qT = boundary(
                d + 1, xsad_ps, master, loc, stc, wq_next)

        # ================= HEAD =================
        msel = pers.tile([128, 256], dt.bfloat16)
        nc.sync.dma_start(out=msel[:], in_=P['MSEL'][:])
        wkct = pers.tile([128, 2048], dt.bfloat16)
        nc.sync.dma_start(out=wkct[:], in_=P['WKCT'][:])
        wem = pers.tile([128, 512], dt.bfloat16)
        nc.sync.dma_start(out=wem[:], in_=P['WEM'][:])
        bkc_sb = pers.tile([128, 8], dt.float32)
        nc.sync.dma_start(out=bkc_sb[:], in_=P['BKC'][:])
        ett = pers.tile([128, 512], dt.bfloat16)
        nc.sync.dma_start(out=ett[:], in_=P['ETT'][:])

        g, s2r, s2r16 = finish_gather_min(ag_out)
        rem = finish_gather_rest(g, s2r, nat)

        def fullsl(kc, kb):
            if kb < 2:
                return loc[:, kc * 256 + kb * 128: kc * 256 + kb * 128 + 128]
            return rem[:, kc * 256 + (kb - 2) * 128: kc * 256 + (kb - 2) * 128 + 128]

        # lptok: (e, j)
        lptok = hp.tile([128, 128], dt.bfloat16, name='lptok')
        for ec in range(2):
            l_ps = pp.tile([128, 64], dt.float32, tag='ps', name='l_ps')
            for kb in range(4):
                mm(l_ps[:], nat[:, kb * 256 + ec * 128: kb * 256 + ec * 128 + 128],
                   msel[:, kb * 64:(kb + 1) * 64],
                   start=(kb == 0), stop=(kb == 3))
            V.tensor_copy(out=lptok[:, ec * 64:(ec + 1) * 64], in_=l_ps[:])

        # xx: kchoice (e, n) n = j*4+kn
        xxsb = hp.tile([128, 512], dt.bfloat16, name='xxsb')
        for kn in range(KN):
            for ec in range(2):
                x_ps = pp.tile([128, 64], dt.float32, tag='ps', name='x_ps')
                for fc in range(2):
                    off = (fc * 8 + kn * 2 + ec) * 128
                    mm(x_ps[:], wkct[:, off:off + 128],
                       lptok[:, fc * 64:(fc + 1) * 64],
                       start=(fc == 0), stop=(fc == 1))
                dst = xxsb[:, ec * 256:(ec + 1) * 256].rearrange(
                    'p (j f) -> p f j', f=4)[:, kn, :]
                V.tensor_scalar_add(
                    out=dst, in0=x_ps[:],
                    scalar1=bkc_sb[:, kn * 2 + ec: kn * 2 + ec + 1])

        # xx2T: (l, n) blocks (core row order)
        xx2 = hp.tile([128, 1024], dt.bfloat16, name='xx2')
        for kb in range(4):
            x_ps = pp.tile([128, 256], dt.float32, tag='ps', name='x2_ps')
            for ec in range(2):
                mm(x_ps[:], fullsl(ec, kb), xxsb[:, ec * 256:(ec + 1) * 256],
                   start=(ec == 0), stop=(ec == 1))
            V.tensor_copy(out=xx2[:, kb * 256:(kb + 1) * 256], in_=x_ps[:])

        # xx3T: (e, n)
        xx3 = hp.tile([128, 512], dt.bfloat16, name='xx3')
        for ec in range(2):
            x_ps = pp.tile([128, 256], dt.float32, tag='ps', name='x3_ps')
            for kb in range(4):
                mm(x_ps[:], nat[:, kb * 256 + ec * 128: kb * 256 + ec * 128 + 128],
                   xx2[:, kb * 256:(kb + 1) * 256],
                   start=(kb == 0), stop=(kb == 3))
            V.tensor_copy(out=xx3[:, ec * 256:(ec + 1) * 256], in_=x_ps[:])

        # xxWT: (e, n) -- local batch only
        xxw = hp.tile([128, 512], dt.bfloat16, name='xxw')
        for ec in range(2):
            x_ps = pp.tile([128, 256], dt.float32, tag='ps', name='xw_ps')
            for kc in range(2):
                mm(x_ps[:], wem[:, (kc * 2 + ec) * 128:(kc * 2 + ec + 1) * 128],
                   xx3[:, kc * 256:(kc + 1) * 256],
                   start=(kc == 0), stop=(kc == 1))
            V.tensor_copy(out=xxw[:, ec * 256:(ec + 1) * 256], in_=x_ps[:])

        # clog: per-row dot of xxW with target embedding
        tb = hp.tile([128, 512], dt.bfloat16, name='tb')
        for ec in range(2):
            V.tensor_mul(tb[:, ec * 256:(ec + 1) * 256],
                         xxw[:, ec * 256:(ec + 1) * 256],
                         ett[:, ec * 256:(ec + 1) * 256])
        cl_ps = pps.tile([1, 256], dt.float32, tag='pss', name='cl_ps')
        for ec in range(2):
            mm(cl_ps[:], ones_cb[:], tb[:, ec * 256:(ec + 1) * 256],
               start=(ec == 0), stop=(ec == 1))
        cl_sb = hp.tile([1, 256], dt.float32, name='cl_sb')
        V.tensor_copy(out=cl_sb[:], in_=cl_ps[:])
        nc.sync.dma_start(out=oclog_t[:], in_=cl_sb[:])

        # logits + per-row sum-exp over own half-vocab
        stats = hp.tile([128, 64], dt.float32, name='stats')
        for vc in range(NVC):
            embt = hp.tile([128, 1000], dt.float8e4, tag='embt', bufs=6,
                           name='embt')
            nc.sync.dma_start(out=embt[:], in_=P['EMBT'][vc])
            for nb in range(2):
                lg_ps = pp.tile([128, VC], dt.float32, tag='ps', name='lg_ps')
                for ec in range(2):
                    mm(lg_ps[:], xxw[:, ec * 256 + nb * 128: ec * 256 + nb * 128 + 128],
                       embt[:, ec * VC:(ec + 1) * VC],
                       start=(ec == 0), stop=(ec == 1))
                escr = hp.tile([128, VC], dt.bfloat16, tag='escr', bufs=1,
                               name='escr')
                act(out=escr[:], in_=lg_ps[:], func=AF.Exp,
                    accum_out=stats[:, nb * 32 + vc: nb * 32 + vc + 1])
        AX = mybir.AxisListType
        sef = hp.tile([128, 2], dt.float32, name='sef')
        for nb in range(2):
            V.reduce_sum(out=sef[:, nb:nb + 1],
                         in_=stats[:, nb * 32:(nb + 1) * 32], axis=AX.X)
        nc.sync.dma_start(out=osum_t[:], in_=sef[:])

    nc.compile()
    return nc


def kernel(**inputs):
    from concourse.bass_utils import run_bass_kernel_spmd

    in_maps, aux = _prep(inputs)
    key = (_D_EFF, _DEBUG)
    if key not in _CACHE:
        _CACHE[key] = _build(_D_EFF, _DEBUG)
    nc = _CACHE[key]
    res = run_bass_kernel_spmd(nc, in_maps, list(range(NCORES)), trace=_TRACE)
    kernel._last_results = res
    summer = np.asarray(aux['summer'], np.float64)

    loss = np.zeros(B, np.float64)
    for b in range(B):
        S = np.zeros(256, np.float64)
        for h in range(2):
            st = np.asarray(res.results[2 * b + h]['osum'], np.float64)  # [128,2]
            for nb in range(2):
                S[nb * 128:(nb + 1) * 128] += st[:, nb]
        cl = np.asarray(res.results[2 * b]['oclog'], np.float64).reshape(256)
        k_lp = (cl - np.log(S)).reshape(M, KN)
        mx = k_lp.max(-1, keepdims=True)
        lp = np.log(np.exp(k_lp - mx).sum(-1)) + mx[:, 0] - np.log(KN)
        sw = summer[b].sum()
        loss[b] = -(lp * summer[b]).sum() / max(sw, 1.0)
    return loss.astype(np.float32)
